# revision 1
# baseline (speedup 1.0000x reference)
"""GPT-J attention (B=2, S=2048, D=4096, 16 heads x 256, partial RoPE 64) on 8 trn2 cores.

Sharding: tensor-parallel over heads — each core owns 2 heads (Wq/Wk/Wv column
slices, Wo row slice), computes its partial out-projection, on-device
ReduceScatter sums partials and leaves each core with a 512-row shard of the
[B*S, D] output; host concatenates shards.

Device kernel layout strategy (per core):
  - hidden_states transposed on host to hsT [B, D, S] so the model dim (the
    matmul contraction) lies on SBUF partitions.
  - QKV projection: out = W_chunk.T @ hsT_chunk accumulated over d-chunks.
    Weights are streamed in NG d-groups; partial sums accumulated in SBUF.
  - QT/KT produced feature-major [hd, s]; V produced natural [s, hd] (by
    swapping stationary/moving operands) so PV can use V as stationary.
  - RoPE: rot = C * q + S2 * swap(q); swap(q) via a tiny PE matmul with a
    pair-swap permutation matrix; C/S2 precomputed on host, laid out [64, S].
  - Attention per (b, head): flash-style over 512-query macro tiles, scores
    via PE, exp on ACT (fused 1/16 scale; causal mask added to diagonal
    tiles from host-built additive mask patterns), unnormalized P transposed
    via PE (identity) to feed PV; softmax normalization deferred to the
    out-projection evacuation (tensor_scalar by 1/rowsum, rowsum collected
    free via activation accum_out).
  - Out-projection: y_partial = attnT.T @ WoT accumulated over local hd,
    normalized at PSUM evacuation, ReduceScatter(add) across 8 cores.

All matmuls run in float32r (fp32 bits, replicated PE mode: full rate at
free-dim >= 256) unless MM_DTYPE says otherwise.
"""

import os
import sys

import numpy as np

sys.path.insert(0, "/opt/trn_rl_repo")

# ---------------------------------------------------------------- constants
B = 2
S = 2048
D = 4096
NH = 16
HD = 256
ROT = 64
MAX_POS = 2048
N_CORES = 8
HPC = NH // N_CORES          # heads per core = 2
HDL = HPC * HD               # local head width = 512

SC = 512                     # s-chunk (projection, q-macro, k-tile width)
QS = 128                     # q-subtile
NEG = -1.0e30


def _cfg_full():
    return dict(B=B, S=S, D=D, HPC=HPC, HD=HD, ROT=ROT)


# ---------------------------------------------------------------- bass build

def build_nc(cfg, use_collective=True, n_cores=N_CORES, mm_dtype="float32r", debug_taps=False):
    import concourse.tile as tile
    from concourse import bacc, mybir

    fp32 = mybir.dt.float32
    mdt = getattr(mybir.dt, mm_dtype)

    Bc, Sc, Dc, HPCc, HDc, ROTc = (
        cfg["B"], cfg["S"], cfg["D"], cfg["HPC"], cfg["HD"], cfg["ROT"])
    HDLc = HPCc * HDc                    # local head width
    NHC = HDLc // 128                    # local hd chunks (4)
    NSC = Sc // SC                       # s-chunks (4)
    NDC = Dc // 128                      # d chunks (32)
    DG = 1024 if Dc % 1024 == 0 else Dc  # d-group size
    NG = Dc // DG                        # d-groups
    GDC = DG // 128                      # d-chunks per group (8)
    SHARD = (Bc * Sc) // n_cores if use_collective else Bc * Sc

    nc = bacc.Bacc(num_devices=n_cores)

    # inputs (per-core)
    hsT_e = nc.declare_dram_parameter("hsT", [Bc, Dc, Sc], mdt, isOutput=False)
    wqT_e = nc.declare_dram_parameter("wqT", [Dc, HDLc], mdt, isOutput=False)
    wkT_e = nc.declare_dram_parameter("wkT", [Dc, HDLc], mdt, isOutput=False)
    wvT_e = nc.declare_dram_parameter("wvT", [Dc, HDLc], mdt, isOutput=False)
    woT_e = nc.declare_dram_parameter("woT", [HDLc, Dc], mdt, isOutput=False)
    cos_e = nc.declare_dram_parameter("cosb", [Bc, ROTc, Sc], mdt, isOutput=False)
    sin_e = nc.declare_dram_parameter("sinb", [Bc, ROTc, Sc], mdt, isOutput=False)
    msk_e = nc.declare_dram_parameter("masks", [QS, 4, SC], fp32, isOutput=False)
    psw_e = nc.declare_dram_parameter("pswap", [128, ROTc], mdt, isOutput=False)
    idn_e = nc.declare_dram_parameter("ident", [128, 128], fp32, isOutput=False)

    y_e = nc.declare_dram_parameter("y", [SHARD, Dc], fp32, isOutput=True)
    if debug_taps:
        dbg = {}
        for nm, shp in [("dbg_qt", [NHC, 128, Sc]), ("dbg_kt", [NHC, 128, Sc]),
                        ("dbg_v", [Sc, HDLc]), ("dbg_atn", [NHC, 128, Sc]),
                        ("dbg_recip", [128, HPCc, Sc // QS])]:
            dbg[nm] = nc.declare_dram_parameter(nm, shp, fp32, isOutput=True)
    if use_collective:
        y_part = nc.dram_tensor("y_part", [Bc * Sc, Dc], fp32)
        rs_out = nc.dram_tensor("rs_out", [SHARD, Dc], fp32)

    def mm(ps, lhsT, rhs, start, stop):
        nc.tensor.matmul(ps, lhsT, rhs, start=start, stop=stop)

    with tile.TileContext(nc) as tc:
        with tc.tile_pool(name="const", bufs=1) as constp:
            masks = constp.tile([QS, 4, SC], fp32)
            nc.sync.dma_start(masks[:], msk_e[:])
            pswap = constp.tile([128, ROTc], mdt)
            nc.sync.dma_start(pswap[:], psw_e[:])
            ident = constp.tile([128, 128], fp32)
            nc.sync.dma_start(ident[:], idn_e[:])

            with (
                tc.tile_pool(name="qkv", bufs=1) as qkvp,
                tc.tile_pool(name="atn", bufs=1) as atnp,
                tc.tile_pool(name="rsum", bufs=1) as rsump,
            ):
                for b in range(Bc):
                    # persistent per-b tensors (slots reused across b)
                    QT = [qkvp.tile([128, Sc], mdt, tag=f"QT{c}", name=f"QT{c}") for c in range(NHC)]
                    KT = [qkvp.tile([128, Sc], mdt, tag=f"KT{c}", name=f"KT{c}") for c in range(NHC)]
                    V = [qkvp.tile([128, HDLc], mdt, tag=f"V{ss}", name=f"V{ss}")
                         for ss in range(Sc // 128)]
                    recip = rsump.tile([128, HPCc, Sc // QS], fp32, tag="recip")

                    # ---------------- phase A: QKV projection ----------------
                    with (
                        tc.tile_pool(name="wts", bufs=1) as wp,
                        tc.tile_pool(name="hst", bufs=2) as hp,
                        tc.tile_pool(name="pjps", bufs=1, space="PSUM") as pjps,
                    ):
                        for g in range(NG):
                            gsl = slice(g * DG, (g + 1) * DG)
                            wq = wp.tile([128, GDC, HDLc], mdt, tag="wq")
                            wk = wp.tile([128, GDC, HDLc], mdt, tag="wk")
                            wv = wp.tile([128, GDC, HDLc], mdt, tag="wv")
                            nc.sync.dma_start(
                                wq[:], wqT_e[gsl, :].rearrange("(j p) f -> p j f", p=128))
                            nc.sync.dma_start(
                                wk[:], wkT_e[gsl, :].rearrange("(j p) f -> p j f", p=128))
                            nc.sync.dma_start(
                                wv[:], wvT_e[gsl, :].rearrange("(j p) f -> p j f", p=128))
                            for sc in range(NSC):
                                ssl = slice(sc * SC, (sc + 1) * SC)
                                GH = GDC // 2
                                hst_a = hp.tile([128, GH, SC], mdt, tag="hst")
                                hst_b = hp.tile([128, GH, SC], mdt, tag="hst")
                                g0 = slice(g * DG, g * DG + GH * 128)
                                g1 = slice(g * DG + GH * 128, (g + 1) * DG)
                                nc.sync.dma_start(
                                    hst_a[:],
                                    hsT_e[b, g0, ssl].rearrange("(j p) f -> p j f", p=128))
                                nc.sync.dma_start(
                                    hst_b[:],
                                    hsT_e[b, g1, ssl].rearrange("(j p) f -> p j f", p=128))

                                def hst(dc, _a=hst_a, _b=hst_b, _gh=GH):
                                    return _a if dc < _gh else _b

                                def hsti(dc, _gh=GH):
                                    return dc % _gh
                                # Q/K: accumulate over this group's d-chunks
                                psq = [pjps.tile([128, SC], fp32, tag=f"psq{h}", name=f"psq{h}")
                                       for h in range(NHC)]
                                psk = [pjps.tile([128, SC], fp32, tag=f"psk{h}", name=f"psk{h}")
                                       for h in range(NHC)]
                                for dc in range(GDC):
                                    for h in range(NHC):
                                        hsl = slice(h * 128, (h + 1) * 128)
                                        mm(psq[h][:], wq[:, dc, hsl], hst(dc)[:, hsti(dc), :],
                                           start=(dc == 0), stop=(dc == GDC - 1))
                                        mm(psk[h][:], wk[:, dc, hsl], hst(dc)[:, hsti(dc), :],
                                           start=(dc == 0), stop=(dc == GDC - 1))
                                for h in range(NHC):
                                    if g == 0:
                                        nc.vector.tensor_copy(QT[h][:, ssl], psq[h][:])
                                        nc.vector.tensor_copy(KT[h][:, ssl], psk[h][:])
                                    else:
                                        nc.vector.tensor_add(
                                            QT[h][:, ssl], QT[h][:, ssl], psq[h][:])
                                        nc.vector.tensor_add(
                                            KT[h][:, ssl], KT[h][:, ssl], psk[h][:])
                                # V: stationary = hsT s-subtiles
                                psv = [pjps.tile([128, HDLc], fp32, tag=f"psq{ss}", name=f"psv{ss}")
                                       for ss in range(SC // 128)]
                                for dc in range(GDC):
                                    for ss in range(SC // 128):
                                        ssub = slice(ss * 128, (ss + 1) * 128)
                                        mm(psv[ss][:], hst(dc)[:, hsti(dc), ssub], wv[:, dc, :],
                                           start=(dc == 0), stop=(dc == GDC - 1))
                                for ss in range(SC // 128):
                                    vi = sc * (SC // 128) + ss
                                    if g == 0:
                                        nc.vector.tensor_copy(V[vi][:], psv[ss][:])
                                    else:
                                        nc.vector.tensor_add(V[vi][:], V[vi][:], psv[ss][:])

                    # ---------------- RoPE on QT/KT rot rows ----------------
                    with (
                        tc.tile_pool(name="trig", bufs=1) as trigp,
                        tc.tile_pool(name="rope", bufs=4) as ropep,
                        tc.tile_pool(name="rops", bufs=2, space="PSUM") as ropsp,
                    ):
                        cosb = trigp.tile([ROTc, Sc], mdt, tag="cos")
                        sinb = trigp.tile([ROTc, Sc], mdt, tag="sin")
                        nc.sync.dma_start(cosb[:], cos_e[b])
                        nc.sync.dma_start(sinb[:], sin_e[b])
                        for t in (QT, KT):
                            for hch in range(0, NHC, HDc // 128):
                                for sc in range(NSC):
                                    ssl = slice(sc * SC, (sc + 1) * SC)
                                    sw = ropsp.tile([ROTc, SC], fp32, tag="sw")
                                    mm(sw[:], pswap[:, :], t[hch][:, ssl],
                                       start=True, stop=True)
                                    t1 = ropep.tile([ROTc, SC], mdt, tag="t1")
                                    t2 = ropep.tile([ROTc, SC], mdt, tag="t2")
                                    nc.vector.tensor_tensor(
                                        t1[:], sw[:], sinb[:, ssl],
                                        op=mybir.AluOpType.mult)
                                    nc.vector.tensor_tensor(
                                        t2[:], t[hch][0:ROTc, ssl], cosb[:, ssl],
                                        op=mybir.AluOpType.mult)
                                    nc.vector.tensor_add(t[hch][0:ROTc, ssl],
                                                         t1[:], t2[:])

                    # ---------------- phase B: attention ----------------
                    ATN = [atnp.tile([128, Sc], mdt, tag=f"ATN{c}", name=f"ATN{c}") for c in range(NHC)]
                    with (
                        tc.tile_pool(name="pbuf", bufs=1) as pbufp,
                        tc.tile_pool(name="ptsb", bufs=3) as ptsbp,
                        tc.tile_pool(name="scps", bufs=2, space="PSUM") as scps,
                        tc.tile_pool(name="ptps", bufs=2, space="PSUM") as ptps,
                        tc.tile_pool(name="atps", bufs=2, space="PSUM") as atps,
                        tc.tile_pool(name="rs", bufs=8) as rsp,
                    ):
                        for h in range(HPC):
                            c0 = h * (HDc // 128)      # first hd chunk of head
                            for qm in range(NSC):
                                nkt = qm + 1           # valid k-tiles of 512
                                nkc = nkt * (SC // 128)  # valid k-chunks of 128
                                P = [pbufp.tile([128, Sc], fp32, tag=f"P{qs}", name=f"P{qs}")
                                     for qs in range(SC // QS)]
                                for qs in range(SC // QS):
                                    g = qm * (SC // QS) + qs
                                    qsl = slice(g * QS, (g + 1) * QS)
                                    racc = rsp.tile([128, 1], fp32, tag=f"racc{qs}")
                                    for kt in range(nkt):
                                        ksl = slice(kt * SC, (kt + 1) * SC)
                                        ss = scps.tile([128, SC], fp32, tag="ss")
                                        mm(ss[:], QT[c0][:, qsl], KT[c0][:, ksl],
                                           start=True, stop=False)
                                        mm(ss[:], QT[c0 + 1][:, qsl], KT[c0 + 1][:, ksl],
                                           start=False, stop=True)
                                        if kt == nkt - 1:
                                            nc.vector.tensor_add(
                                                ss[:], ss[:], masks[:, qs, :])
                                        if kt == 0:
                                            acc_ap = racc[:]
                                        else:
                                            rpart = rsp.tile([128, 1], fp32, tag="rpart")
                                            acc_ap = rpart[:]
                                        nc.scalar.activation(
                                            P[qs][:, ksl], ss[:],
                                            mybir.ActivationFunctionType.Exp,
                                            bias=0.0, scale=1.0 / 16.0,
                                            accum_out=acc_ap)
                                        if kt > 0:
                                            nc.vector.tensor_add(
                                                racc[:], racc[:], rpart[:])
                                    nc.vector.reciprocal(
                                        recip[:, h, g:g + 1], racc[:])
                                # transpose P + PV
                                atn_ps = [atps.tile([128, SC], fp32, tag=f"atn{hh}", name=f"atnps{hh}")
                                          for hh in range(HDc // 128)]
                                for kc in range(nkc):
                                    kcl = slice(kc * 128, (kc + 1) * 128)
                                    ptp = ptps.tile([128, SC], fp32, tag="ptp")
                                    for qs in range(SC // QS):
                                        nc.tensor.transpose(
                                            ptp[:, qs * 128:(qs + 1) * 128],
                                            P[qs][:, kcl], ident[:])
                                    pts = ptsbp.tile([128, SC], mdt, tag="pts")
                                    nc.vector.tensor_copy(pts[:], ptp[:])
                                    for hh in range(HDc // 128):
                                        mm(atn_ps[hh][:],
                                           V[kc][:, h * HDc + hh * 128:
                                                 h * HDc + (hh + 1) * 128],
                                           pts[:],
                                           start=(kc == 0), stop=(kc == nkc - 1))
                                for hh in range(HDc // 128):
                                    nc.vector.tensor_copy(
                                        ATN[c0 + hh][:, qm * SC:(qm + 1) * SC],
                                        atn_ps[hh][:])

                    if debug_taps and b == 0:
                        for c in range(NHC):
                            nc.sync.dma_start(dbg["dbg_qt"][c], QT[c][:])
                            nc.sync.dma_start(dbg["dbg_kt"][c], KT[c][:])
                            nc.sync.dma_start(dbg["dbg_atn"][c], ATN[c][:])
                        for ss in range(Sc // 128):
                            nc.sync.dma_start(
                                dbg["dbg_v"][ss * 128:(ss + 1) * 128, :], V[ss][:])
                        nc.sync.dma_start(dbg["dbg_recip"][:], recip[:])

                    # ---------------- phase C: out projection ----------------
                    with (
                        tc.tile_pool(name="wo", bufs=2) as wop,
                        tc.tile_pool(name="ysb", bufs=4) as ysbp,
                        tc.tile_pool(name="yps", bufs=3, space="PSUM") as ypsp,
                    ):
                        NCH = HDc // 128   # chunks per head
                        dst = y_part if use_collective else y_e
                        for oc in range(Dc // SC):
                            ocl = slice(oc * SC, (oc + 1) * SC)
                            woc = wop.tile([128, NHC, SC], mdt, tag="woc")
                            nc.sync.dma_start(
                                woc[:],
                                woT_e[:, ocl].rearrange("(c p) f -> p c f", p=128))
                            for sg in range(Sc // QS):
                                ssl = slice(sg * 128, (sg + 1) * 128)
                                ysb = ysbp.tile([128, SC], fp32, tag="ysb")
                                for h in range(HPCc):
                                    yps = ypsp.tile([128, SC], fp32,
                                                    tag=f"yps{h}", name=f"yps{h}")
                                    for cc in range(NCH):
                                        c = h * NCH + cc
                                        mm(yps[:], ATN[c][:, ssl], woc[:, c, :],
                                           start=(cc == 0), stop=(cc == NCH - 1))
                                    if h == 0:
                                        nc.vector.tensor_scalar(
                                            out=ysb[:], in0=yps[:],
                                            scalar1=recip[:, 0, sg:sg + 1],
                                            scalar2=None,
                                            op0=mybir.AluOpType.mult)
                                    else:
                                        nc.vector.scalar_tensor_tensor(
                                            out=ysb[:], in0=yps[:],
                                            scalar=recip[:, h, sg:sg + 1],
                                            in1=ysb[:],
                                            op0=mybir.AluOpType.mult,
                                            op1=mybir.AluOpType.add)
                                nc.sync.dma_start(
                                    dst[b * Sc + sg * 128:b * Sc + (sg + 1) * 128,
                                        ocl],
                                    ysb[:])

            if use_collective:
                nc.gpsimd.collective_compute(
                    "ReduceScatter",
                    mybir.AluOpType.add,
                    replica_groups=[list(range(n_cores))],
                    ins=[y_part[:]],
                    outs=[rs_out[:]],
                )
                nc.sync.dma_start(y_e[:], rs_out[:])

    nc.compile()
    return nc


# ---------------------------------------------------------------- host prep

def _sinusoidal_np(num_pos, dim):
    inv_freq = 1.0 / (10000.0 ** (np.arange(0, dim, 2, dtype=np.float32) / dim))
    t = np.arange(num_pos, dtype=np.float32)[:, None] * inv_freq[None, :]
    return np.cos(t).astype(np.float32), np.sin(t).astype(np.float32)  # [P, dim//2]


def _host_arrays(hs, Wq, Wk, Wv, Wo, position_ids, cfg, n_cores):
    """Build the shared + per-core input arrays."""
    Bc, Sc, Dc, HPCc, HDc, ROTc = (
        cfg["B"], cfg["S"], cfg["D"], cfg["HPC"], cfg["HD"], cfg["ROT"])
    HDLc = HPCc * HDc
    hsT = np.ascontiguousarray(hs.transpose(0, 2, 1)).astype(np.float32)

    cos_t, sin_t = _sinusoidal_np(max(MAX_POS, Sc), ROTc)   # [P, ROT//2]
    pos = np.asarray(position_ids).astype(np.int64)         # [B, S]
    cosg = cos_t[pos]                                       # [B, S, 32]
    sing = sin_t[pos]
    cosb = np.repeat(cosg.transpose(0, 2, 1), 2, axis=1)    # [B, 64, S]
    sinb_r = np.repeat(sing.transpose(0, 2, 1), 2, axis=1)
    sgn = np.ones((ROTc, 1), np.float32)
    sgn[0::2] = -1.0
    sinb = (sinb_r * sgn).astype(np.float32)
    cosb = np.ascontiguousarray(cosb).astype(np.float32)

    masks = np.zeros((4, QS, SC), np.float32)
    qq = np.arange(QS)[:, None]
    kk = np.arange(SC)[None, :]
    for m in range(4):
        masks[m] = np.where(kk <= m * QS + qq, 0.0, NEG)
    masks = np.ascontiguousarray(masks.transpose(1, 0, 2))  # [QS, 4, SC]

    pswap = np.zeros((128, ROTc), np.float32)
    for f in range(ROTc // 2):
        pswap[2 * f + 1, 2 * f] = 1.0
        pswap[2 * f, 2 * f + 1] = 1.0
    ident = np.eye(128, dtype=np.float32)

    shared = dict(hsT=hsT, cosb=cosb, sinb=sinb, masks=masks,
                  pswap=pswap, ident=ident)
    per_core = []
    for c in range(n_cores):
        csl = slice(c * HDLc, (c + 1) * HDLc)
        per_core.append(dict(
            wqT=np.ascontiguousarray(Wq[csl, :].T),
            wkT=np.ascontiguousarray(Wk[csl, :].T),
            wvT=np.ascontiguousarray(Wv[csl, :].T),
            woT=np.ascontiguousarray(Wo[:, csl].T),
            **shared,
        ))
    return per_core


def _numpy_reference(hidden_states, Wq, Wk, Wv, Wo, layer_past_k, layer_past_v,
                     attention_mask, position_ids, new_key_loc, new_value_loc,
                     valid_key_indices, valid_value_indices, bucket_size):
    """Slow but general fallback (mirrors reference.py in numpy fp32)."""
    hs = np.asarray(hidden_states, np.float32)
    Bc, Sc, Dc = hs.shape
    q = (hs @ np.asarray(Wq).T).reshape(Bc, Sc, NH, HD)
    k = (hs @ np.asarray(Wk).T).reshape(Bc, Sc, NH, HD)
    v = (hs @ np.asarray(Wv).T).reshape(Bc, Sc, NH, HD)

    cos_t, sin_t = _sinusoidal_np(MAX_POS, ROT)
    pos = np.asarray(position_ids).astype(np.int64)
    c_ = cos_t[pos][:, :, None, :]      # [B,S,1,32]
    s_ = sin_t[pos][:, :, None, :]

    def rot(x):
        xr = x[..., :ROT].reshape(Bc, Sc, NH, ROT // 2, 2)
        x0, x1 = xr[..., 0], xr[..., 1]
        o0 = c_ * x0 - s_ * x1
        o1 = s_ * x0 + c_ * x1
        out = np.stack([o0, o1], axis=-1).reshape(Bc, Sc, NH, ROT)
        return np.concatenate([out, x[..., ROT:]], axis=-1)

    q, k = rot(q), rot(k)
    nk = np.asarray(layer_past_k, np.float32).copy()
    nv = np.asarray(layer_past_v, np.float32).copy()
    nk[np.asarray(new_key_loc)] = k.reshape(Bc * Sc, 1, NH, HD)
    nv[np.asarray(new_value_loc)] = v.reshape(Bc * Sc, 1, NH, HD)
    kg = nk[np.asarray(valid_key_indices)].reshape(
        Bc, bucket_size, NH, HD).transpose(0, 2, 1, 3)
    vg = nv[np.asarray(valid_value_indices)].reshape(
        Bc, bucket_size, NH, HD).transpose(0, 2, 1, 3)
    qh = q.transpose(0, 2, 1, 3)
    scores = np.einsum("bhqd,bhkd->bhqk", qh, kg)
    causal = np.tril(np.ones((MAX_POS, MAX_POS), bool))[
        bucket_size - Sc:bucket_size, :bucket_size]
    scores = np.where(causal, scores, np.float32(np.finfo(np.float32).min))
    scores = scores / np.float32(np.sqrt(HD)) + np.asarray(attention_mask, np.float32)
    scores = scores - scores.max(-1, keepdims=True)
    p = np.exp(scores)
    p = p / p.sum(-1, keepdims=True)
    attn = np.einsum("bhqk,bhkd->bhqd", p, vg)
    attn = attn.transpose(0, 2, 1, 3).reshape(Bc, Sc, Dc)
    return (attn @ np.asarray(Wo).T).astype(np.float32)


def _fast_path_ok(layer_past_k, layer_past_v, attention_mask, new_key_loc,
                  new_value_loc, valid_key_indices, valid_value_indices,
                  bucket_size, hs_shape):
    Bc, Sc, Dc = hs_shape
    if (Bc, Sc, Dc) != (B, S, D) or int(bucket_size) != S:
        return False
    ar = np.arange(Bc * Sc)
    for idx in (new_key_loc, new_value_loc, valid_key_indices, valid_value_indices):
        a = np.asarray(idx)
        if a.shape != (Bc * Sc,) or not np.array_equal(a, ar):
            return False
    if np.any(np.asarray(attention_mask) != 0):
        return False
    return True


_NC_CACHE = {}


def _get_nc(use_collective=True):
    key = ("full", use_collective)
    if key not in _NC_CACHE:
        _NC_CACHE[key] = build_nc(_cfg_full(), use_collective=use_collective,
                                  n_cores=N_CORES)
    return _NC_CACHE[key]


def kernel(**inputs):
    hs = np.asarray(inputs["hidden_states"], np.float32)
    fast = _fast_path_ok(
        inputs["layer_past_k"], inputs["layer_past_v"], inputs["attention_mask"],
        inputs["new_key_loc"], inputs["new_value_loc"],
        inputs["valid_key_indices"], inputs["valid_value_indices"],
        inputs["bucket_size"], hs.shape)
    if not fast:
        return _numpy_reference(**inputs)

    from concourse.bass_utils import run_bass_kernel_spmd

    use_collective = os.environ.get("KERNEL_NO_COLLECTIVE", "") != "1"
    nc = _get_nc(use_collective)
    in_maps = _host_arrays(
        hs, np.asarray(inputs["Wq"], np.float32),
        np.asarray(inputs["Wk"], np.float32),
        np.asarray(inputs["Wv"], np.float32),
        np.asarray(inputs["Wo"], np.float32),
        inputs["position_ids"], _cfg_full(), N_CORES)
    res = run_bass_kernel_spmd(nc, in_maps, list(range(N_CORES)))
    outs = [res.results[c]["y"] for c in range(N_CORES)]
    if use_collective:
        y = np.concatenate(outs, axis=0)
    else:
        y = np.sum(np.stack(outs), axis=0)
    return y.reshape(B, S, D).astype(np.float32)



# revision 8
# speedup vs baseline: 1.7144x; 1.7144x over previous
"""GPT-J attention (B=2, S=2048, D=4096, 16 heads x 256, partial RoPE 64) on 8 trn2 cores.

Sharding: tensor-parallel over heads for QKV+attention (each core owns 2 heads:
Wq/Wk/Wv column slices), then an AllToAll converts head-sharding into
sequence-sharding so the out-projection runs with the FULL Wo on a 512-token
shard per core — no ReduceScatter of the 64 MiB partial outputs (the A2A moves
only ~4 MiB of bf16 attention outputs). Each core emits a disjoint
[512, 4096] fp32 output shard; host concatenates.

Device kernel (per core), all matmuls bf16 (fp32 PSUM accumulate):
  - hidden_states pre-transposed on host to hsT [B, D, S] bf16 (contraction on
    partitions).
  - QKV projection per (b, 512-token chunk): three passes (Q, K, V), each
    accumulating all 32 d-chunks directly in PSUM (start/stop over the full
    contraction); weights streamed in 1024-row quarters, hst chunk resident.
    QT/KT produced feature-major [hd, s]; V token-major [s, hd] (stationary /
    moving swapped).
  - RoPE on rot rows via pair-swap PE matmul + DVE mul/add (host-built
    cos/sin with sign folded in), applied per s-chunk right after projection.
  - Attention per (b, head): scores computed TRANSPOSED (ssT [k,q] tiles:
    stationary=KT chunk, moving=QT 512-wide q block) so exp output PT [k, q]
    feeds PV directly as the moving operand (no P transposes, no transpose
    evacuation copies). Causal masks added on diagonal k-chunks (host-built
    transposed patterns). Row sums via a ones[128,128] stationary matmul
    accumulated in PSUM (gives the sum broadcast across partitions for free);
    softmax normalization applied during the PV PSUM->SBUF evacuation
    (tensor_tensor multiply by reciprocal), then DMA straight to the A2A
    input buffer.
  - AllToAll [8 blocks of 512 feat x 512 tok] -> each core holds all 4096
    attention features for its 512 tokens, feature-major.
  - Out-projection: y[tok128, of512] tiles, stationary = z feature chunks,
    moving = full-Wo column blocks streamed (32 MiB bf16, double buffered).
"""

import os
import sys

import numpy as np

sys.path.insert(0, "/opt/trn_rl_repo")

# ---------------------------------------------------------------- constants
B = 2
S = 2048
D = 4096
NH = 16
HD = 256
ROT = 64
MAX_POS = 2048
N_CORES = 8
HPC = NH // N_CORES          # heads per core = 2
HDL = HPC * HD               # local head width = 512

SC = 512                     # s-chunk (projection, q-macro width)
NEG = -1.0e30


def _cfg_full():
    return dict(B=B, S=S, D=D, HPC=HPC, HD=HD, ROT=ROT)


# ---------------------------------------------------------------- bass build

def build_nc(cfg, use_collective=True, n_cores=N_CORES, mm_dtype="bfloat16"):
    import concourse.tile as tile
    from concourse import bacc, mybir

    fp32 = mybir.dt.float32
    bdt = getattr(mybir.dt, mm_dtype)

    Bc, Sc, Dc, HPCc, HDc, ROTc = (
        cfg["B"], cfg["S"], cfg["D"], cfg["HPC"], cfg["HD"], cfg["ROT"])
    HDLc = HPCc * HDc                    # local head width (512)
    NHC = HDLc // 128                    # local hd chunks (4)
    NSC = Sc // SC                       # s-chunks (4)
    NKC = Sc // 128                      # k-chunks per batch (16)
    NQ = 4                               # d-quarters (1024 rows each)
    DCQ = Dc // NQ // 128                # d-chunks per quarter (8)
    SHARD = (Bc * Sc) // n_cores         # tokens per core after A2A (512)
    NTT = SHARD // 128                   # token tiles per core (4)
    NFC = Dc // 128                      # feature chunks (32)
    NOB = Dc // SC                       # out-proj 512-wide blocks (8)

    nc = bacc.Bacc(num_devices=n_cores)

    # inputs (per-core)
    hsT_e = nc.declare_dram_parameter("hsT", [Bc, Dc, Sc], bdt, isOutput=False)
    wqT_e = nc.declare_dram_parameter("wqT", [Dc, HDLc], bdt, isOutput=False)
    wkT_e = nc.declare_dram_parameter("wkT", [Dc, HDLc], bdt, isOutput=False)
    wvT_e = nc.declare_dram_parameter("wvT", [Dc, HDLc], bdt, isOutput=False)
    woT_e = nc.declare_dram_parameter("woT", [Dc, Dc], bdt, isOutput=False)
    cos_e = nc.declare_dram_parameter("cosb", [Bc, ROTc, Sc], bdt, isOutput=False)
    sin_e = nc.declare_dram_parameter("sinb", [Bc, ROTc, Sc], bdt, isOutput=False)
    msk_e = nc.declare_dram_parameter("masksT", [128, 4, SC], fp32, isOutput=False)
    psw_e = nc.declare_dram_parameter("pswap", [128, ROTc], bdt, isOutput=False)
    one_e = nc.declare_dram_parameter("ones", [128, 128], bdt, isOutput=False)

    y_e = nc.declare_dram_parameter("y", [SHARD, Dc], fp32, isOutput=True)

    # A2A staging: block j = [512 local feats, 512 toks of global token-chunk j]
    yatt = nc.dram_tensor("yatt", [n_cores * HDLc, SC], bdt)
    zatt = nc.dram_tensor("zatt", [n_cores * HDLc, SC], bdt)

    def mm(ps, lhsT, rhs, start, stop):
        nc.tensor.matmul(ps, lhsT, rhs, start=start, stop=stop)

    with tile.TileContext(nc) as tc:
        with tc.tile_pool(name="const", bufs=1) as constp:
            masksT = constp.tile([128, 4, SC], fp32)
            nc.sync.dma_start(masksT[:], msk_e[:])
            pswap = constp.tile([128, ROTc], bdt)
            nc.sync.dma_start(pswap[:], psw_e[:])
            ones_t = constp.tile([128, 128], bdt)
            nc.sync.dma_start(ones_t[:], one_e[:])

            with (
                tc.tile_pool(name="qkv", bufs=1) as qkvp,
            ):
                for b in range(Bc):
                    QT = [qkvp.tile([128, Sc], bdt, tag=f"QT{c}", name=f"QT{c}")
                          for c in range(NHC)]
                    KT = [qkvp.tile([128, Sc], bdt, tag=f"KT{c}", name=f"KT{c}")
                          for c in range(NHC)]
                    V = [qkvp.tile([128, HDLc], bdt, tag=f"V{k}", name=f"V{k}")
                         for k in range(NKC)]

                    # ---------------- phase A: QKV projection + RoPE ----------------
                    with (
                        tc.tile_pool(name="trig", bufs=1) as trigp,
                        tc.tile_pool(name="hst", bufs=6) as hstp,
                        tc.tile_pool(name="wst", bufs=3) as wstp,
                        tc.tile_pool(name="rope", bufs=2) as ropep,
                        tc.tile_pool(name="pjps", bufs=2, space="PSUM") as pjps,
                    ):
                        cosb = trigp.tile([ROTc, Sc], bdt, tag="cos")
                        sinb = trigp.tile([ROTc, Sc], bdt, tag="sin")
                        nc.sync.dma_start(cosb[:], cos_e[b])
                        nc.sync.dma_start(sinb[:], sin_e[b])
                        for sc in range(NSC):
                            ssl = slice(sc * SC, (sc + 1) * SC)
                            hq = []
                            for q in range(NQ):
                                ht = hstp.tile([128, DCQ, SC], bdt, tag="h",
                                               name=f"h{q}")
                                nc.sync.dma_start(
                                    ht[:],
                                    hsT_e[b, q * 1024:(q + 1) * 1024, ssl]
                                    .rearrange("(j p) f -> p j f", p=128))
                                hq.append(ht)

                            # Q and K passes: out [hd=128, s=512] per head-chunk
                            for w_e, T in ((wqT_e, QT), (wkT_e, KT)):
                                ps = [pjps.tile([128, SC], fp32, tag=f"pj{i}",
                                                name=f"pj{i}")
                                      for i in range(NHC)]
                                for q in range(NQ):
                                    wt = wstp.tile([128, DCQ, HDLc], bdt, tag="w")
                                    nc.sync.dma_start(
                                        wt[:],
                                        w_e[q * 1024:(q + 1) * 1024, :]
                                        .rearrange("(j p) f -> p j f", p=128))
                                    for dc in range(DCQ):
                                        for hc in range(NHC):
                                            mm(ps[hc][:],
                                               wt[:, dc, hc * 128:(hc + 1) * 128],
                                               hq[q][:, dc, :],
                                               start=(q == 0 and dc == 0),
                                               stop=(q == NQ - 1 and dc == DCQ - 1))
                                for hc in range(NHC):
                                    nc.scalar.copy(T[hc][:, ssl], ps[hc][:])

                            # V pass: out [s=128, hdl=512] per token subtile
                            ps = [pjps.tile([128, HDLc], fp32, tag=f"pj{i}",
                                            name=f"pv{i}")
                                  for i in range(NHC)]
                            for q in range(NQ):
                                wt = wstp.tile([128, DCQ, HDLc], bdt, tag="w")
                                nc.sync.dma_start(
                                    wt[:],
                                    wvT_e[q * 1024:(q + 1) * 1024, :]
                                    .rearrange("(j p) f -> p j f", p=128))
                                for dc in range(DCQ):
                                    for ts in range(4):
                                        mm(ps[ts][:],
                                           hq[q][:, dc, ts * 128:(ts + 1) * 128],
                                           wt[:, dc, :],
                                           start=(q == 0 and dc == 0),
                                           stop=(q == NQ - 1 and dc == DCQ - 1))
                            for ts in range(4):
                                nc.vector.tensor_copy(V[sc * 4 + ts][:], ps[ts][:])

                            # RoPE on rot rows of this s-chunk (sw reuses a
                            # projection PSUM slot — V pass is done with it)
                            for ti, T in enumerate((QT, KT)):
                                for hch in range(0, NHC, HDc // 128):
                                    sw = pjps.tile([ROTc, SC], fp32,
                                                   tag=f"pj{ti * 2 + hch // 2}",
                                                   name="sw")
                                    mm(sw[:], pswap[:, :], T[hch][:, ssl],
                                       start=True, stop=True)
                                    t1 = ropep.tile([ROTc, SC], bdt, tag="t1")
                                    t2 = ropep.tile([ROTc, SC], bdt, tag="t2")
                                    nc.vector.tensor_tensor(
                                        t1[:], sw[:], sinb[:, ssl],
                                        op=mybir.AluOpType.mult)
                                    nc.vector.tensor_tensor(
                                        t2[:], T[hch][0:ROTc, ssl], cosb[:, ssl],
                                        op=mybir.AluOpType.mult)
                                    nc.vector.tensor_add(T[hch][0:ROTc, ssl],
                                                         t1[:], t2[:])

                    # ---------------- phase B: attention ----------------
                    with (
                        tc.tile_pool(name="ptsb", bufs=5) as ptp,
                        tc.tile_pool(name="rcp", bufs=2) as rcpp,
                        tc.tile_pool(name="atnsb", bufs=2) as atnp,
                        tc.tile_pool(name="ssps", bufs=4, space="PSUM") as ssps,
                        tc.tile_pool(name="atps", bufs=1, space="PSUM") as atps,
                        tc.tile_pool(name="rsps", bufs=1, space="PSUM") as rsps,
                    ):
                        for h in range(HPCc):
                            c0, c1 = 2 * h, 2 * h + 1
                            for qm in range(NSC):
                                nkc = 4 * (qm + 1)
                                qsl = slice(qm * SC, (qm + 1) * SC)
                                rs_ps = rsps.tile([128, SC], fp32, tag="rs")
                                at_ps = [atps.tile([128, SC], fp32, tag=f"at{i}",
                                                   name=f"at{i}")
                                         for i in range(2)]
                                pts = {}
                                # software-pipelined by 2: scores for kc+2
                                # issue before rowsum/PV for kc
                                for step in range(nkc + 2):
                                    if step < nkc:
                                        kc = step
                                        kcl = slice(kc * 128, (kc + 1) * 128)
                                        ss = ssps.tile([128, SC], fp32, tag="ss")
                                        mm(ss[:], KT[c0][:, kcl], QT[c0][:, qsl],
                                           start=True, stop=False)
                                        mm(ss[:], KT[c1][:, kcl], QT[c1][:, qsl],
                                           start=False, stop=True)
                                        if kc >= nkc - 4:
                                            nc.vector.tensor_add(
                                                ss[:], ss[:],
                                                masksT[:, kc - (nkc - 4), :])
                                        pt = ptp.tile([128, SC], bdt, tag="pt")
                                        nc.scalar.activation(
                                            pt[:], ss[:],
                                            mybir.ActivationFunctionType.Exp,
                                            bias=0.0, scale=1.0 / 16.0)
                                        pts[kc] = pt
                                    if step >= 2:
                                        kc = step - 2
                                        pt = pts.pop(kc)
                                        st = (kc == 0)
                                        sp = (kc == nkc - 1)
                                        mm(rs_ps[:], ones_t[:], pt[:],
                                           start=st, stop=sp)
                                        mm(at_ps[0][:],
                                           V[kc][:, h * HDc:h * HDc + 128],
                                           pt[:], start=st, stop=sp)
                                        mm(at_ps[1][:],
                                           V[kc][:, h * HDc + 128:(h + 1) * HDc],
                                           pt[:], start=st, stop=sp)
                                recip = rcpp.tile([128, SC], fp32, tag="rc")
                                nc.vector.reciprocal(recip[:], rs_ps[:])
                                for hh in range(2):
                                    atn = atnp.tile([128, SC], bdt, tag=f"atn{hh}")
                                    nc.vector.tensor_tensor(
                                        atn[:], at_ps[hh][:], recip[:],
                                        op=mybir.AluOpType.mult)
                                    row0 = h * HDc + hh * 128
                                    nc.sync.dma_start(
                                        yatt[(b * NSC + qm) * HDLc + row0:
                                             (b * NSC + qm) * HDLc + row0 + 128, :],
                                        atn[:])

            # ---------------- A2A: head-sharded -> token-sharded ----------------
            nc.gpsimd.collective_compute(
                "AllToAll",
                mybir.AluOpType.bypass,
                replica_groups=[list(range(n_cores))],
                ins=[yatt[:]],
                outs=[zatt[:]],
            )

            # ---------------- phase C: out projection ----------------
            with (
                tc.tile_pool(name="zsb", bufs=1) as zp,
                tc.tile_pool(name="wo", bufs=2) as wop,
                tc.tile_pool(name="ysb", bufs=4) as ysbp,
                tc.tile_pool(name="yps", bufs=3, space="PSUM") as ypsp,
            ):
                z = []
                for q in range(NQ):
                    zt = zp.tile([128, DCQ, SHARD], bdt, tag=f"z{q}", name=f"z{q}")
                    nc.sync.dma_start(
                        zt[:],
                        zatt[q * 1024:(q + 1) * 1024, :]
                        .rearrange("(j p) f -> p j f", p=128))
                    z.append(zt)
                for ob in range(NOB):
                    ocl = slice(ob * SC, (ob + 1) * SC)
                    wo_t = wop.tile([128, NFC, SC], bdt, tag="wo")
                    nc.sync.dma_start(
                        wo_t[:],
                        woT_e[:, ocl].rearrange("(j p) f -> p j f", p=128))
                    for tt in range(NTT):
                        tsl = slice(tt * 128, (tt + 1) * 128)
                        yp = ypsp.tile([128, SC], fp32, tag="yp")
                        for fc in range(NFC):
                            mm(yp[:], z[fc // DCQ][:, fc % DCQ, tsl],
                               wo_t[:, fc, :],
                               start=(fc == 0), stop=(fc == NFC - 1))
                        ysb = ysbp.tile([128, SC], fp32, tag="ysb")
                        nc.scalar.copy(ysb[:], yp[:])
                        nc.sync.dma_start(y_e[tsl, ocl], ysb[:])

    nc.compile()
    return nc


# ---------------------------------------------------------------- host prep

def _sinusoidal_np(num_pos, dim):
    inv_freq = 1.0 / (10000.0 ** (np.arange(0, dim, 2, dtype=np.float32) / dim))
    t = np.arange(num_pos, dtype=np.float32)[:, None] * inv_freq[None, :]
    return np.cos(t).astype(np.float32), np.sin(t).astype(np.float32)  # [P, dim//2]


def _host_arrays(hs, Wq, Wk, Wv, Wo, position_ids, cfg, n_cores):
    """Build the shared + per-core input arrays."""
    import ml_dtypes
    bf16 = ml_dtypes.bfloat16

    Bc, Sc, Dc, HPCc, HDc, ROTc = (
        cfg["B"], cfg["S"], cfg["D"], cfg["HPC"], cfg["HD"], cfg["ROT"])
    HDLc = HPCc * HDc
    hsT = np.ascontiguousarray(hs.transpose(0, 2, 1)).astype(bf16)

    cos_t, sin_t = _sinusoidal_np(max(MAX_POS, Sc), ROTc)   # [P, ROT//2]
    pos = np.asarray(position_ids).astype(np.int64)         # [B, S]
    cosg = cos_t[pos]                                       # [B, S, 32]
    sing = sin_t[pos]
    cosb = np.repeat(cosg.transpose(0, 2, 1), 2, axis=1)    # [B, 64, S]
    sinb_r = np.repeat(sing.transpose(0, 2, 1), 2, axis=1)
    sgn = np.ones((ROTc, 1), np.float32)
    sgn[0::2] = -1.0
    sinb = np.ascontiguousarray(sinb_r * sgn).astype(bf16)
    cosb = np.ascontiguousarray(cosb).astype(bf16)

    # transposed causal patterns for the 4 diagonal k-chunks of a 512 q-block:
    # masksT[r, m, c] = 0 if (m*128 + r) <= c else NEG   (k_local <= q_local)
    kk = np.arange(128)[:, None, None]
    mm_ = np.arange(4)[None, :, None]
    qq = np.arange(SC)[None, None, :]
    masksT = np.where(mm_ * 128 + kk <= qq, 0.0, NEG).astype(np.float32)

    pswap = np.zeros((128, ROTc), np.float32)
    for f in range(ROTc // 2):
        pswap[2 * f + 1, 2 * f] = 1.0
        pswap[2 * f, 2 * f + 1] = 1.0
    pswap = pswap.astype(bf16)
    ones = np.ones((128, 128), np.float32).astype(bf16)

    woT = np.ascontiguousarray(np.asarray(Wo, np.float32).T).astype(bf16)

    shared = dict(hsT=hsT, cosb=cosb, sinb=sinb, masksT=masksT,
                  pswap=pswap, ones=ones, woT=woT)
    per_core = []
    for c in range(n_cores):
        csl = slice(c * HDLc, (c + 1) * HDLc)
        per_core.append(dict(
            wqT=np.ascontiguousarray(Wq[csl, :].T).astype(bf16),
            wkT=np.ascontiguousarray(Wk[csl, :].T).astype(bf16),
            wvT=np.ascontiguousarray(Wv[csl, :].T).astype(bf16),
            **shared,
        ))
    return per_core


def _numpy_reference(hidden_states, Wq, Wk, Wv, Wo, layer_past_k, layer_past_v,
                     attention_mask, position_ids, new_key_loc, new_value_loc,
                     valid_key_indices, valid_value_indices, bucket_size):
    """Slow but general fallback (mirrors reference.py in numpy fp32)."""
    hs = np.asarray(hidden_states, np.float32)
    Bc, Sc, Dc = hs.shape
    q = (hs @ np.asarray(Wq).T).reshape(Bc, Sc, NH, HD)
    k = (hs @ np.asarray(Wk).T).reshape(Bc, Sc, NH, HD)
    v = (hs @ np.asarray(Wv).T).reshape(Bc, Sc, NH, HD)

    cos_t, sin_t = _sinusoidal_np(MAX_POS, ROT)
    pos = np.asarray(position_ids).astype(np.int64)
    c_ = cos_t[pos][:, :, None, :]      # [B,S,1,32]
    s_ = sin_t[pos][:, :, None, :]

    def rot(x):
        xr = x[..., :ROT].reshape(Bc, Sc, NH, ROT // 2, 2)
        x0, x1 = xr[..., 0], xr[..., 1]
        o0 = c_ * x0 - s_ * x1
        o1 = s_ * x0 + c_ * x1
        out = np.stack([o0, o1], axis=-1).reshape(Bc, Sc, NH, ROT)
        return np.concatenate([out, x[..., ROT:]], axis=-1)

    q, k = rot(q), rot(k)
    nk = np.asarray(layer_past_k, np.float32).copy()
    nv = np.asarray(layer_past_v, np.float32).copy()
    nk[np.asarray(new_key_loc)] = k.reshape(Bc * Sc, 1, NH, HD)
    nv[np.asarray(new_value_loc)] = v.reshape(Bc * Sc, 1, NH, HD)
    kg = nk[np.asarray(valid_key_indices)].reshape(
        Bc, bucket_size, NH, HD).transpose(0, 2, 1, 3)
    vg = nv[np.asarray(valid_value_indices)].reshape(
        Bc, bucket_size, NH, HD).transpose(0, 2, 1, 3)
    qh = q.transpose(0, 2, 1, 3)
    scores = np.einsum("bhqd,bhkd->bhqk", qh, kg)
    causal = np.tril(np.ones((MAX_POS, MAX_POS), bool))[
        bucket_size - Sc:bucket_size, :bucket_size]
    scores = np.where(causal, scores, np.float32(np.finfo(np.float32).min))
    scores = scores / np.float32(np.sqrt(HD)) + np.asarray(attention_mask, np.float32)
    scores = scores - scores.max(-1, keepdims=True)
    p = np.exp(scores)
    p = p / p.sum(-1, keepdims=True)
    attn = np.einsum("bhqk,bhkd->bhqd", p, vg)
    attn = attn.transpose(0, 2, 1, 3).reshape(Bc, Sc, Dc)
    return (attn @ np.asarray(Wo).T).astype(np.float32)


def _fast_path_ok(layer_past_k, layer_past_v, attention_mask, new_key_loc,
                  new_value_loc, valid_key_indices, valid_value_indices,
                  bucket_size, hs_shape):
    Bc, Sc, Dc = hs_shape
    if (Bc, Sc, Dc) != (B, S, D) or int(bucket_size) != S:
        return False
    ar = np.arange(Bc * Sc)
    for idx in (new_key_loc, new_value_loc, valid_key_indices, valid_value_indices):
        a = np.asarray(idx)
        if a.shape != (Bc * Sc,) or not np.array_equal(a, ar):
            return False
    if np.any(np.asarray(attention_mask) != 0):
        return False
    return True


_NC_CACHE = {}


def _get_nc(use_collective=True):
    key = "full"
    if key not in _NC_CACHE:
        _NC_CACHE[key] = build_nc(_cfg_full(), n_cores=N_CORES)
    return _NC_CACHE[key]


def kernel(**inputs):
    hs = np.asarray(inputs["hidden_states"], np.float32)
    fast = _fast_path_ok(
        inputs["layer_past_k"], inputs["layer_past_v"], inputs["attention_mask"],
        inputs["new_key_loc"], inputs["new_value_loc"],
        inputs["valid_key_indices"], inputs["valid_value_indices"],
        inputs["bucket_size"], hs.shape)
    if not fast:
        return _numpy_reference(**inputs)

    from concourse.bass_utils import run_bass_kernel_spmd

    nc = _get_nc(True)
    in_maps = _host_arrays(
        hs, np.asarray(inputs["Wq"], np.float32),
        np.asarray(inputs["Wk"], np.float32),
        np.asarray(inputs["Wv"], np.float32),
        np.asarray(inputs["Wo"], np.float32),
        inputs["position_ids"], _cfg_full(), N_CORES)
    res = run_bass_kernel_spmd(nc, in_maps, list(range(N_CORES)))
    outs = [res.results[c]["y"] for c in range(N_CORES)]
    y = np.concatenate(outs, axis=0)
    return y.reshape(B, S, D).astype(np.float32)


# revision 11
# speedup vs baseline: 1.7984x; 1.0490x over previous
"""GPT-J attention (B=2, S=2048, D=4096, 16 heads x 256, partial RoPE 64) on 8 trn2 cores.

Sharding: tensor-parallel over heads for QKV+attention (each core owns 2 heads:
Wq/Wk/Wv column slices), then an AllToAll converts head-sharding into
sequence-sharding so the out-projection runs with the FULL Wo on a 512-token
shard per core — no ReduceScatter of the 64 MiB partial outputs (the A2A moves
only ~4 MiB of bf16 attention outputs). Each core emits a disjoint
[512, 4096] fp32 output shard; host concatenates.

Device kernel (per core), all matmuls bf16 (fp32 PSUM accumulate):
  - hidden_states pre-transposed on host to hsT [B, D, S] bf16 (contraction on
    partitions).
  - QKV projection per (b, 512-token chunk): three passes (Q, K, V), each
    accumulating all 32 d-chunks directly in PSUM (start/stop over the full
    contraction); weights streamed in 1024-row quarters, hst chunk resident.
    QT/KT produced feature-major [hd, s]; V token-major [s, hd] (stationary /
    moving swapped).
  - RoPE on rot rows via pair-swap PE matmul + DVE mul/add (host-built
    cos/sin with sign folded in), applied per s-chunk right after projection.
  - Attention per (b, head): scores computed TRANSPOSED (ssT [k,q] tiles:
    stationary=KT chunk, moving=QT 512-wide q block) so exp output PT [k, q]
    feeds PV directly as the moving operand (no P transposes, no transpose
    evacuation copies). Causal masks added on diagonal k-chunks (host-built
    transposed patterns). Row sums via a ones[128,128] stationary matmul
    accumulated in PSUM (gives the sum broadcast across partitions for free);
    softmax normalization applied during the PV PSUM->SBUF evacuation
    (tensor_tensor multiply by reciprocal), then DMA straight to the A2A
    input buffer.
  - AllToAll [8 blocks of 512 feat x 512 tok] -> each core holds all 4096
    attention features for its 512 tokens, feature-major.
  - Out-projection: y[tok128, of512] tiles, stationary = z feature chunks,
    moving = full-Wo column blocks streamed (32 MiB bf16, double buffered).
"""

import os
import sys

import numpy as np

sys.path.insert(0, "/opt/trn_rl_repo")

# ---------------------------------------------------------------- constants
B = 2
S = 2048
D = 4096
NH = 16
HD = 256
ROT = 64
MAX_POS = 2048
N_CORES = 8
HPC = NH // N_CORES          # heads per core = 2
HDL = HPC * HD               # local head width = 512

SC = 512                     # s-chunk (projection, q-macro width)
NEG = -1.0e30


def _cfg_full():
    return dict(B=B, S=S, D=D, HPC=HPC, HD=HD, ROT=ROT)


# ---------------------------------------------------------------- bass build

def build_nc(cfg, use_collective=True, n_cores=N_CORES, mm_dtype="bfloat16"):
    import concourse.tile as tile
    from concourse import bacc, mybir

    fp32 = mybir.dt.float32
    bdt = getattr(mybir.dt, mm_dtype)

    Bc, Sc, Dc, HPCc, HDc, ROTc = (
        cfg["B"], cfg["S"], cfg["D"], cfg["HPC"], cfg["HD"], cfg["ROT"])
    HDLc = HPCc * HDc                    # local head width (512)
    NHC = HDLc // 128                    # local hd chunks (4)
    NSC = Sc // SC                       # s-chunks (4)
    NKC = Sc // 128                      # k-chunks per batch (16)
    NQ = 4                               # d-quarters (1024 rows each)
    DCQ = Dc // NQ // 128                # d-chunks per quarter (8)
    SHARD = (Bc * Sc) // n_cores         # tokens per core after A2A (512)
    NTT = SHARD // 128                   # token tiles per core (4)
    NFC = Dc // 128                      # feature chunks (32)
    NOB = Dc // SC                       # out-proj 512-wide blocks (8)

    nc = bacc.Bacc(num_devices=n_cores)

    # inputs (per-core)
    hsT_e = nc.declare_dram_parameter("hsT", [Bc, Dc, Sc], bdt, isOutput=False)
    wqT_e = nc.declare_dram_parameter("wqT", [Dc, HDLc], bdt, isOutput=False)
    wkT_e = nc.declare_dram_parameter("wkT", [Dc, HDLc], bdt, isOutput=False)
    wvT_e = nc.declare_dram_parameter("wvT", [Dc, HDLc], bdt, isOutput=False)
    woT_e = nc.declare_dram_parameter("woT", [Dc, Dc], bdt, isOutput=False)
    cos_e = nc.declare_dram_parameter("cosb", [Bc, ROTc, Sc], bdt, isOutput=False)
    sin_e = nc.declare_dram_parameter("sinb", [Bc, ROTc, Sc], bdt, isOutput=False)
    msk_e = nc.declare_dram_parameter("masksT", [128, 4, SC], fp32, isOutput=False)
    psw_e = nc.declare_dram_parameter("pswap", [128, ROTc], bdt, isOutput=False)
    one_e = nc.declare_dram_parameter("ones", [128, 128], bdt, isOutput=False)

    y_e = nc.declare_dram_parameter("y", [SHARD, Dc], fp32, isOutput=True)

    # A2A staging per batch: block j = [512 local feats, 256 toks of batch-b
    # token-chunk j]. Splitting per batch lets A2A(b0) overlap b1 compute.
    TPB = SHARD // Bc                    # tokens per core per batch (256)
    yatt = [nc.dram_tensor(f"yatt{b}", [n_cores * HDLc, TPB], bdt)
            for b in range(Bc)]
    zatt = [nc.dram_tensor(f"zatt{b}", [n_cores * HDLc, TPB], bdt)
            for b in range(Bc)]

    def mm(ps, lhsT, rhs, start, stop):
        nc.tensor.matmul(ps, lhsT, rhs, start=start, stop=stop)

    with tile.TileContext(nc) as tc:
        with tc.tile_pool(name="const", bufs=1) as constp:
            masksT = constp.tile([128, 4, SC], fp32)
            nc.sync.dma_start(masksT[:], msk_e[:])
            pswap = constp.tile([128, ROTc], bdt)
            nc.sync.dma_start(pswap[:], psw_e[:])
            ones_t = constp.tile([128, 128], bdt)
            nc.sync.dma_start(ones_t[:], one_e[:])
            cosb = [constp.tile([ROTc, Sc], bdt, name=f"cos{b}") for b in range(Bc)]
            sinb = [constp.tile([ROTc, Sc], bdt, name=f"sin{b}") for b in range(Bc)]
            for b in range(Bc):
                nc.sync.dma_start(cosb[b][:], cos_e[b])
                nc.sync.dma_start(sinb[b][:], sin_e[b])

            with (
                tc.tile_pool(name="qkv", bufs=1) as qkvp,
                tc.tile_pool(name="hst", bufs=9) as hstp,
                tc.tile_pool(name="wst", bufs=3) as wstp,
                tc.tile_pool(name="rope", bufs=2) as ropep,
                tc.tile_pool(name="ptsb", bufs=8) as ptp,
                tc.tile_pool(name="rcp", bufs=2) as rcpp,
                tc.tile_pool(name="atnsb", bufs=2) as atnp,
            ):
                def load_hq(b, sc):
                    ssl = slice(sc * SC, (sc + 1) * SC)
                    hq = []
                    for q in range(NQ):
                        ht = hstp.tile([128, DCQ, SC], bdt, tag="h", name=f"h{q}")
                        nc.sync.dma_start(
                            ht[:],
                            hsT_e[b, q * 1024:(q + 1) * 1024, ssl]
                            .rearrange("(j p) f -> p j f", p=128))
                        hq.append(ht)
                    return hq

                next_hq = load_hq(0, 0)
                for b in range(Bc):
                    QT = [qkvp.tile([128, Sc], bdt, tag=f"QT{c}", name=f"QT{c}")
                          for c in range(NHC)]
                    KT = [qkvp.tile([128, Sc], bdt, tag=f"KT{c}", name=f"KT{c}")
                          for c in range(NHC)]
                    V = [qkvp.tile([128, HDLc], bdt, tag=f"V{k}", name=f"V{k}")
                         for k in range(NKC)]

                    # ---------------- phase A: QKV projection + RoPE ----------------
                    with tc.tile_pool(name="pjps", bufs=2, space="PSUM") as pjps:
                        for sc in range(NSC):
                            ssl = slice(sc * SC, (sc + 1) * SC)
                            hq = next_hq
                            if sc + 1 < NSC:
                                next_hq = load_hq(b, sc + 1)
                            elif b + 1 < Bc:
                                next_hq = load_hq(b + 1, 0)

                            # Q and K passes: out [hd=128, s=512] per head-chunk
                            for w_e, T in ((wqT_e, QT), (wkT_e, KT)):
                                ps = [pjps.tile([128, SC], fp32, tag=f"pj{i}",
                                                name=f"pj{i}")
                                      for i in range(NHC)]
                                for q in range(NQ):
                                    wt = wstp.tile([128, DCQ, HDLc], bdt, tag="w")
                                    nc.sync.dma_start(
                                        wt[:],
                                        w_e[q * 1024:(q + 1) * 1024, :]
                                        .rearrange("(j p) f -> p j f", p=128))
                                    for dc in range(DCQ):
                                        for hc in range(NHC):
                                            mm(ps[hc][:],
                                               wt[:, dc, hc * 128:(hc + 1) * 128],
                                               hq[q][:, dc, :],
                                               start=(q == 0 and dc == 0),
                                               stop=(q == NQ - 1 and dc == DCQ - 1))
                                for hc in range(NHC):
                                    nc.scalar.copy(T[hc][:, ssl], ps[hc][:])

                            # V pass: out [s=128, hdl=512] per token subtile
                            ps = [pjps.tile([128, HDLc], fp32, tag=f"pj{i}",
                                            name=f"pv{i}")
                                  for i in range(NHC)]
                            for q in range(NQ):
                                wt = wstp.tile([128, DCQ, HDLc], bdt, tag="w")
                                nc.sync.dma_start(
                                    wt[:],
                                    wvT_e[q * 1024:(q + 1) * 1024, :]
                                    .rearrange("(j p) f -> p j f", p=128))
                                for dc in range(DCQ):
                                    for ts in range(4):
                                        mm(ps[ts][:],
                                           hq[q][:, dc, ts * 128:(ts + 1) * 128],
                                           wt[:, dc, :],
                                           start=(q == 0 and dc == 0),
                                           stop=(q == NQ - 1 and dc == DCQ - 1))
                            for ts in range(4):
                                nc.vector.tensor_copy(V[sc * 4 + ts][:], ps[ts][:])

                            # RoPE on rot rows of this s-chunk (sw reuses a
                            # projection PSUM slot — V pass is done with it)
                            for ti, T in enumerate((QT, KT)):
                                for hch in range(0, NHC, HDc // 128):
                                    sw = pjps.tile([ROTc, SC], fp32,
                                                   tag=f"pj{ti * 2 + hch // 2}",
                                                   name="sw")
                                    mm(sw[:], pswap[:, :], T[hch][:, ssl],
                                       start=True, stop=True)
                                    t1 = ropep.tile([ROTc, SC], bdt, tag="t1")
                                    t2 = ropep.tile([ROTc, SC], bdt, tag="t2")
                                    nc.vector.tensor_tensor(
                                        t1[:], sw[:], sinb[b][:, ssl],
                                        op=mybir.AluOpType.mult)
                                    nc.vector.tensor_tensor(
                                        t2[:], T[hch][0:ROTc, ssl], cosb[b][:, ssl],
                                        op=mybir.AluOpType.mult)
                                    nc.vector.tensor_add(T[hch][0:ROTc, ssl],
                                                         t1[:], t2[:])

                    # ---------------- phase B: attention ----------------
                    with (
                        tc.tile_pool(name="ssps", bufs=4, space="PSUM") as ssps,
                        tc.tile_pool(name="atps", bufs=1, space="PSUM") as atps,
                        tc.tile_pool(name="rsps", bufs=1, space="PSUM") as rsps,
                    ):
                        for h in range(HPCc):
                            c0, c1 = 2 * h, 2 * h + 1
                            for qm in range(NSC):
                                nkc = 4 * (qm + 1)
                                npair = nkc // 2
                                LAG = 2          # rowsum/PV lag in kc-pairs
                                qsl = slice(qm * SC, (qm + 1) * SC)
                                rs_ps = rsps.tile([128, SC], fp32, tag="rs")
                                at_ps = [atps.tile([128, SC], fp32, tag=f"at{i}",
                                                   name=f"at{i}")
                                         for i in range(2)]
                                pts = {}
                                # kc processed in pairs, banks interleaved so
                                # no two consecutive matmuls hit the same
                                # PSUM bank; rowsum/PV lag LAG pairs behind
                                # so exp (ACT) latency stays off the PE
                                # critical path.
                                for step in range(npair + LAG):
                                    if step < npair:
                                        k0, k1 = 2 * step, 2 * step + 1
                                        l0 = slice(k0 * 128, (k0 + 1) * 128)
                                        l1 = slice(k1 * 128, (k1 + 1) * 128)
                                        s0 = ssps.tile([128, SC], fp32, tag="ss",
                                                       name="ss0")
                                        s1 = ssps.tile([128, SC], fp32, tag="ss",
                                                       name="ss1")
                                        mm(s0[:], KT[c0][:, l0], QT[c0][:, qsl],
                                           start=True, stop=False)
                                        mm(s1[:], KT[c0][:, l1], QT[c0][:, qsl],
                                           start=True, stop=False)
                                        mm(s0[:], KT[c1][:, l0], QT[c1][:, qsl],
                                           start=False, stop=True)
                                        mm(s1[:], KT[c1][:, l1], QT[c1][:, qsl],
                                           start=False, stop=True)
                                        for kc, ss in ((k0, s0), (k1, s1)):
                                            if kc >= nkc - 4:
                                                nc.vector.tensor_add(
                                                    ss[:], ss[:],
                                                    masksT[:, kc - (nkc - 4), :])
                                            pt = ptp.tile([128, SC], bdt, tag="pt")
                                            nc.scalar.activation(
                                                pt[:], ss[:],
                                                mybir.ActivationFunctionType.Exp,
                                                bias=0.0, scale=1.0 / 16.0)
                                            pts[kc] = pt
                                    if step >= LAG:
                                        for kc in (2 * (step - LAG),
                                                   2 * (step - LAG) + 1):
                                            pt = pts.pop(kc)
                                            st = (kc == 0)
                                            sp = (kc == nkc - 1)
                                            mm(rs_ps[:], ones_t[:], pt[:],
                                               start=st, stop=sp)
                                            mm(at_ps[0][:],
                                               V[kc][:, h * HDc:h * HDc + 128],
                                               pt[:], start=st, stop=sp)
                                            mm(at_ps[1][:],
                                               V[kc][:, h * HDc + 128:(h + 1) * HDc],
                                               pt[:], start=st, stop=sp)
                                recip = rcpp.tile([128, SC], fp32, tag="rc")
                                nc.vector.reciprocal(recip[:], rs_ps[:])
                                for hh in range(2):
                                    atn = atnp.tile([128, SC], bdt, tag=f"atn{hh}")
                                    nc.vector.tensor_tensor(
                                        atn[:], at_ps[hh][:], recip[:],
                                        op=mybir.AluOpType.mult)
                                    row0 = h * HDc + hh * 128
                                    # split across the two destination cores
                                    # covering this 512-token q block
                                    for half in range(2):
                                        j = 2 * qm + half
                                        nc.sync.dma_start(
                                            yatt[b][j * HDLc + row0:
                                                    j * HDLc + row0 + 128, :],
                                            atn[:, half * TPB:(half + 1) * TPB])

                    # A2A(b): head-sharded -> token-sharded; b0's overlaps
                    # b1's QKV/attention compute entirely.
                    nc.gpsimd.collective_compute(
                        "AllToAll",
                        mybir.AluOpType.bypass,
                        replica_groups=[list(range(n_cores))],
                        ins=[yatt[b][:]],
                        outs=[zatt[b][:]],
                    )

            # ---------------- phase C: out projection ----------------
            with (
                tc.tile_pool(name="zsb", bufs=1) as zp,
                tc.tile_pool(name="wo", bufs=2) as wop,
                tc.tile_pool(name="ysb", bufs=4) as ysbp,
                tc.tile_pool(name="yps", bufs=4, space="PSUM") as ypsp,
            ):
                z = []
                for q in range(NQ):
                    zt = zp.tile([128, DCQ, SHARD], bdt, tag=f"z{q}", name=f"z{q}")
                    for b in range(Bc):
                        nc.sync.dma_start(
                            zt[:, :, b * TPB:(b + 1) * TPB],
                            zatt[b][q * 1024:(q + 1) * 1024, :]
                            .rearrange("(j p) f -> p j f", p=128))
                    z.append(zt)
                for ob in range(NOB):
                    ocl = slice(ob * SC, (ob + 1) * SC)
                    wo_t = wop.tile([128, NFC, SC], bdt, tag="wo")
                    nc.sync.dma_start(
                        wo_t[:],
                        woT_e[:, ocl].rearrange("(j p) f -> p j f", p=128))
                    # 4 token-tile accumulation chains interleaved so
                    # consecutive matmuls hit different PSUM banks
                    yp = [ypsp.tile([128, SC], fp32, tag="yp", name=f"yp{tt}")
                          for tt in range(NTT)]
                    for fc in range(NFC):
                        for tt in range(NTT):
                            mm(yp[tt][:],
                               z[fc // DCQ][:, fc % DCQ,
                                            tt * 128:(tt + 1) * 128],
                               wo_t[:, fc, :],
                               start=(fc == 0), stop=(fc == NFC - 1))
                    for tt in range(NTT):
                        tsl = slice(tt * 128, (tt + 1) * 128)
                        ysb = ysbp.tile([128, SC], fp32, tag="ysb")
                        nc.scalar.copy(ysb[:], yp[tt][:])
                        nc.sync.dma_start(y_e[tsl, ocl], ysb[:])

    nc.compile()
    return nc


# ---------------------------------------------------------------- host prep

def _sinusoidal_np(num_pos, dim):
    inv_freq = 1.0 / (10000.0 ** (np.arange(0, dim, 2, dtype=np.float32) / dim))
    t = np.arange(num_pos, dtype=np.float32)[:, None] * inv_freq[None, :]
    return np.cos(t).astype(np.float32), np.sin(t).astype(np.float32)  # [P, dim//2]


def _host_arrays(hs, Wq, Wk, Wv, Wo, position_ids, cfg, n_cores):
    """Build the shared + per-core input arrays."""
    import ml_dtypes
    bf16 = ml_dtypes.bfloat16

    Bc, Sc, Dc, HPCc, HDc, ROTc = (
        cfg["B"], cfg["S"], cfg["D"], cfg["HPC"], cfg["HD"], cfg["ROT"])
    HDLc = HPCc * HDc
    hsT = np.ascontiguousarray(hs.transpose(0, 2, 1)).astype(bf16)

    cos_t, sin_t = _sinusoidal_np(max(MAX_POS, Sc), ROTc)   # [P, ROT//2]
    pos = np.asarray(position_ids).astype(np.int64)         # [B, S]
    cosg = cos_t[pos]                                       # [B, S, 32]
    sing = sin_t[pos]
    cosb = np.repeat(cosg.transpose(0, 2, 1), 2, axis=1)    # [B, 64, S]
    sinb_r = np.repeat(sing.transpose(0, 2, 1), 2, axis=1)
    sgn = np.ones((ROTc, 1), np.float32)
    sgn[0::2] = -1.0
    sinb = np.ascontiguousarray(sinb_r * sgn).astype(bf16)
    cosb = np.ascontiguousarray(cosb).astype(bf16)

    # transposed causal patterns for the 4 diagonal k-chunks of a 512 q-block:
    # masksT[r, m, c] = 0 if (m*128 + r) <= c else NEG   (k_local <= q_local)
    kk = np.arange(128)[:, None, None]
    mm_ = np.arange(4)[None, :, None]
    qq = np.arange(SC)[None, None, :]
    masksT = np.where(mm_ * 128 + kk <= qq, 0.0, NEG).astype(np.float32)

    pswap = np.zeros((128, ROTc), np.float32)
    for f in range(ROTc // 2):
        pswap[2 * f + 1, 2 * f] = 1.0
        pswap[2 * f, 2 * f + 1] = 1.0
    pswap = pswap.astype(bf16)
    ones = np.ones((128, 128), np.float32).astype(bf16)

    woT = np.ascontiguousarray(np.asarray(Wo, np.float32).T).astype(bf16)

    shared = dict(hsT=hsT, cosb=cosb, sinb=sinb, masksT=masksT,
                  pswap=pswap, ones=ones, woT=woT)
    per_core = []
    for c in range(n_cores):
        csl = slice(c * HDLc, (c + 1) * HDLc)
        per_core.append(dict(
            wqT=np.ascontiguousarray(Wq[csl, :].T).astype(bf16),
            wkT=np.ascontiguousarray(Wk[csl, :].T).astype(bf16),
            wvT=np.ascontiguousarray(Wv[csl, :].T).astype(bf16),
            **shared,
        ))
    return per_core


def _numpy_reference(hidden_states, Wq, Wk, Wv, Wo, layer_past_k, layer_past_v,
                     attention_mask, position_ids, new_key_loc, new_value_loc,
                     valid_key_indices, valid_value_indices, bucket_size):
    """Slow but general fallback (mirrors reference.py in numpy fp32)."""
    hs = np.asarray(hidden_states, np.float32)
    Bc, Sc, Dc = hs.shape
    q = (hs @ np.asarray(Wq).T).reshape(Bc, Sc, NH, HD)
    k = (hs @ np.asarray(Wk).T).reshape(Bc, Sc, NH, HD)
    v = (hs @ np.asarray(Wv).T).reshape(Bc, Sc, NH, HD)

    cos_t, sin_t = _sinusoidal_np(MAX_POS, ROT)
    pos = np.asarray(position_ids).astype(np.int64)
    c_ = cos_t[pos][:, :, None, :]      # [B,S,1,32]
    s_ = sin_t[pos][:, :, None, :]

    def rot(x):
        xr = x[..., :ROT].reshape(Bc, Sc, NH, ROT // 2, 2)
        x0, x1 = xr[..., 0], xr[..., 1]
        o0 = c_ * x0 - s_ * x1
        o1 = s_ * x0 + c_ * x1
        out = np.stack([o0, o1], axis=-1).reshape(Bc, Sc, NH, ROT)
        return np.concatenate([out, x[..., ROT:]], axis=-1)

    q, k = rot(q), rot(k)
    nk = np.asarray(layer_past_k, np.float32).copy()
    nv = np.asarray(layer_past_v, np.float32).copy()
    nk[np.asarray(new_key_loc)] = k.reshape(Bc * Sc, 1, NH, HD)
    nv[np.asarray(new_value_loc)] = v.reshape(Bc * Sc, 1, NH, HD)
    kg = nk[np.asarray(valid_key_indices)].reshape(
        Bc, bucket_size, NH, HD).transpose(0, 2, 1, 3)
    vg = nv[np.asarray(valid_value_indices)].reshape(
        Bc, bucket_size, NH, HD).transpose(0, 2, 1, 3)
    qh = q.transpose(0, 2, 1, 3)
    scores = np.einsum("bhqd,bhkd->bhqk", qh, kg)
    causal = np.tril(np.ones((MAX_POS, MAX_POS), bool))[
        bucket_size - Sc:bucket_size, :bucket_size]
    scores = np.where(causal, scores, np.float32(np.finfo(np.float32).min))
    scores = scores / np.float32(np.sqrt(HD)) + np.asarray(attention_mask, np.float32)
    scores = scores - scores.max(-1, keepdims=True)
    p = np.exp(scores)
    p = p / p.sum(-1, keepdims=True)
    attn = np.einsum("bhqk,bhkd->bhqd", p, vg)
    attn = attn.transpose(0, 2, 1, 3).reshape(Bc, Sc, Dc)
    return (attn @ np.asarray(Wo).T).astype(np.float32)


def _fast_path_ok(layer_past_k, layer_past_v, attention_mask, new_key_loc,
                  new_value_loc, valid_key_indices, valid_value_indices,
                  bucket_size, hs_shape):
    Bc, Sc, Dc = hs_shape
    if (Bc, Sc, Dc) != (B, S, D) or int(bucket_size) != S:
        return False
    ar = np.arange(Bc * Sc)
    for idx in (new_key_loc, new_value_loc, valid_key_indices, valid_value_indices):
        a = np.asarray(idx)
        if a.shape != (Bc * Sc,) or not np.array_equal(a, ar):
            return False
    if np.any(np.asarray(attention_mask) != 0):
        return False
    return True


_NC_CACHE = {}


def _get_nc(use_collective=True):
    key = "full"
    if key not in _NC_CACHE:
        _NC_CACHE[key] = build_nc(_cfg_full(), n_cores=N_CORES)
    return _NC_CACHE[key]


def kernel(**inputs):
    hs = np.asarray(inputs["hidden_states"], np.float32)
    fast = _fast_path_ok(
        inputs["layer_past_k"], inputs["layer_past_v"], inputs["attention_mask"],
        inputs["new_key_loc"], inputs["new_value_loc"],
        inputs["valid_key_indices"], inputs["valid_value_indices"],
        inputs["bucket_size"], hs.shape)
    if not fast:
        return _numpy_reference(**inputs)

    from concourse.bass_utils import run_bass_kernel_spmd

    nc = _get_nc(True)
    in_maps = _host_arrays(
        hs, np.asarray(inputs["Wq"], np.float32),
        np.asarray(inputs["Wk"], np.float32),
        np.asarray(inputs["Wv"], np.float32),
        np.asarray(inputs["Wo"], np.float32),
        inputs["position_ids"], _cfg_full(), N_CORES)
    res = run_bass_kernel_spmd(nc, in_maps, list(range(N_CORES)))
    outs = [res.results[c]["y"] for c in range(N_CORES)]
    return _unshard(outs)


def _unshard(outs):
    """Core c's [512, D] shard = [b0 tokens 256c:256(c+1); b1 same range]."""
    tpb = (B * S) // N_CORES // B        # 256
    y = np.empty((B, S, D), np.float32)
    for c, o in enumerate(outs):
        for b in range(B):
            y[b, c * tpb:(c + 1) * tpb] = o[b * tpb:(b + 1) * tpb]
    return y
